# revision 7
# baseline (speedup 1.0000x reference)
"""Trainium2 Bass kernel: bilinear grid_sample (align_corners=True).

reference: coord [N,2] in [-1,1], params [1,32,1024,1024] -> out [N,32].

Strategy (8 NeuronCores, grid-sharded with per-cell dedup):
  - Host: build an fp16 "diff quad" table [H*W, 128]: cell (y,x) holds,
    channel-major, (a, dx, dy, dxy) = (v00, v01-v00, v10-v00,
    v11-v10-v01+v00) per channel (256B/cell), so the device combine is
    out = a + fx*dx + fy*dy + (fx*fy)*dxy with host-precomputed weights.
  - Sharding: core c owns grid bands [4c, 4c+4) of 32 rows each (the
    in-band cell index (y0&31)*1024+x0 must fit int16 for dma_gather);
    every query is routed to the core owning its band.  ~250k queries
    per core at ~1.9 queries/cell density.
  - Dedup: dma_gather descriptor generation (Q7-pair-serial, ~6ns/desc)
    is the bottleneck, so queries sharing a cell are paired: one
    256B descriptor serves 2 queries (region A, dup-2 slots); odd
    remainders go to region C (1 slot/desc).  1.256M descs for 2M
    queries.  Region gathers are split in 2 subs to fit the 16384-desc
    SWDGE ring and round-robin over all 4 SWDGE queues (4 Q7 core pairs
    generate concurrently, ~3.6x).
  - Compute per sub-gather, chunked over columns: ACT dense-expands the
    3 weights over channels plane by plane (DVE broadcast-operand mults
    are ~7x slower than dense); DVE multiplies each plane (region A
    reads gathered quads through a stride-0 "dup" dim so both slots of
    a pair share one fetch) and sums the 4 planes.  HWDGE (nc.sync)
    handles regular loads/stores so Pool only runs gather desc-gen.
  - Host de-permutes the padded fp16 outputs back to query order, fp32.
"""

import os
import sys

import numpy as np

for _p in ("/opt/trn_rl_repo",):
    if os.path.isdir(_p) and _p not in sys.path:
        sys.path.insert(0, _p)

from contextlib import ExitStack

import concourse.tile as tile
from concourse import bacc, bass, mybir
from concourse.bass_utils import run_bass_kernel_spmd
from concourse.library_config import mlp

F16 = mybir.dt.float16
F32 = mybir.dt.float32
I16 = mybir.dt.int16

N_POINTS = 2_000_000
C = 32
H = 1024
W = 1024
QUAD = 4 * C  # 128 fp16 elems = 256B per table cell
N_CORES = 8

BANDS = 32
ROWS_PER_BAND = H // BANDS  # 32
BAND_CELLS = ROWS_PER_BAND * W  # 32768 -> in-band idx fits int16
BPC = BANDS // N_CORES  # 4 bands per core
SUBS = 2  # sub-gathers per region per band (SWDGE ring limit 16384)
CAP_A = 11776  # pair descs per A-sub (92*128); observed max band 23538/2
CAP_C = 8192  # single descs per C-sub (64*128); observed max band 16213/2
P = 128
NQ = 4  # SWDGE queues: one Q7 core pair each
KCH_A = 46  # compute chunk columns (kdA=92 -> 2 chunks)
KCH_C = 32


def build_program(capA: int, capC: int, repeat: int = 1):
    assert capA % 128 == 0 and capC % 128 == 0
    kdA, kdC = capA // 128, capC // 128
    sbA, sbC = capA // 16, capC // 16

    nc = bacc.Bacc(
        "TRN2",
        target_bir_lowering=False,
        debug=False,
        num_devices=N_CORES,
        num_swdge_queues=NQ,
    )
    MUL, ADD = mybir.AluOpType.mult, mybir.AluOpType.add
    COPY = mybir.ActivationFunctionType.Copy
    NSUB = BPC * SUBS  # row-blocks per region

    table_t = nc.dram_tensor(
        "table", [BPC * BAND_CELLS, QUAD], F16, kind="ExternalInput"
    )
    idxA_t = nc.dram_tensor("idxA", [NSUB * P, sbA], I16, kind="ExternalInput")
    idxC_t = nc.dram_tensor("idxC", [NSUB * P, sbC], I16, kind="ExternalInput")
    fA_t = nc.dram_tensor("fA", [NSUB * P, kdA * 2 * 3], F16, kind="ExternalInput")
    fC_t = nc.dram_tensor("fC", [NSUB * P, kdC * 3], F16, kind="ExternalInput")
    outA_t = nc.dram_tensor("outA", [NSUB * P, kdA * 2 * C], F16, kind="ExternalOutput")
    outC_t = nc.dram_tensor("outC", [NSUB * P, kdC * C], F16, kind="ExternalOutput")

    with tile.TileContext(nc) as tc, ExitStack() as ctx:
        nc.gpsimd.load_library(mlp)
        in_pool = ctx.enter_context(tc.tile_pool(name="in", bufs=3))
        gA_pool = ctx.enter_context(tc.tile_pool(name="gA", bufs=2))
        gC_pool = ctx.enter_context(tc.tile_pool(name="gC", bufs=2))
        oA_pool = ctx.enter_context(tc.tile_pool(name="oA", bufs=2))
        oC_pool = ctx.enter_context(tc.tile_pool(name="oC", bufs=2))
        wd_pool = ctx.enter_context(tc.tile_pool(name="wd", bufs=3))
        m_pool = ctx.enter_context(tc.tile_pool(name="m", bufs=4))
        s_pool = ctx.enter_context(tc.tile_pool(name="s", bufs=2))

        table_ap = table_t.ap()
        qrr = 0  # queue round-robin

        def region(lb, sub, reg, rep):
            nonlocal qrr
            cap, kd, sb_, kch = (
                (capA, kdA, sbA, KCH_A) if reg == "A" else (capC, kdC, sbC, KCH_C)
            )
            dup = 2 if reg == "A" else 1
            idx_ap = (idxA_t if reg == "A" else idxC_t).ap()
            f_ap = (fA_t if reg == "A" else fC_t).ap()
            out_ap = (outA_t if reg == "A" else outC_t).ap()
            g_pool = gA_pool if reg == "A" else gC_pool
            o_pool = oA_pool if reg == "A" else oC_pool
            r0 = (lb * SUBS + sub) * P

            nm = f"{reg}{lb}{sub}r{rep}"
            idx_s = in_pool.tile([P, sb_], I16, tag=f"idx{reg}", name=f"ix{nm}")
            f_s = in_pool.tile([P, kd * dup * 3], F16, tag=f"f{reg}", name=f"f{nm}")
            nc.sync.dma_start(out=idx_s[:], in_=idx_ap[r0 : r0 + P, :])
            nc.sync.dma_start(out=f_s[:], in_=f_ap[r0 : r0 + P, :])

            g = g_pool.tile([P, kd * QUAD], F16, tag="g", name=f"g{nm}")
            g3 = g[:].rearrange("p (k e) -> p k e", e=QUAD)
            nc.gpsimd.dma_gather(
                g3,
                table_ap[lb * BAND_CELLS : (lb + 1) * BAND_CELLS, :],
                idx_s[:],
                cap,
                cap,
                QUAD,
                single_packet=False,
                queue_num=qrr % NQ,
            )
            qrr += 1

            g4 = g[:].rearrange("p (k c j) -> p k c j", c=C, j=4)
            f4 = f_s[:].rearrange("p (k d j) -> p k d j", d=dup, j=3)
            o = o_pool.tile([P, kd * dup * C], F16, tag="o", name=f"o{nm}")
            o4 = o[:].rearrange("p (k d c) -> p k d c", d=dup, c=C)

            for c0 in range(0, kd, kch):
                c1 = min(c0 + kch, kd)
                w = c1 - c0
                gq = g4[:, c0:c1]  # [p, w, C, 4]
                ms = []
                for j in range(3):
                    wd = wd_pool.tile(
                        [P, kch * dup * C], F16, tag="wd", name=f"wd{nm}c{c0}j{j}"
                    )
                    wdv = wd[:, : w * dup * C].rearrange(
                        "p (k d c) -> p k d c", d=dup, c=C
                    )
                    fb = f4[:, c0:c1, :, j : j + 1].to_broadcast([P, w, dup, C])
                    nc.scalar.activation(wdv, fb, COPY)
                    m = m_pool.tile(
                        [P, kch * dup * C], F16, tag="m", name=f"m{nm}c{c0}j{j}"
                    )
                    mv = m[:, : w * dup * C].rearrange(
                        "p (k d c) -> p k d c", d=dup, c=C
                    )
                    gp = gq[:, :, :, j + 1 : j + 2].squeeze(3)  # [p, w, C] stride 4
                    if dup == 2:
                        gp = gp.unsqueeze(2).to_broadcast([P, w, 2, C])
                    else:
                        gp = gp.unsqueeze(2)
                    nc.vector.tensor_tensor(out=mv, in0=gp, in1=wdv, op=MUL)
                    ms.append(mv)
                s = s_pool.tile(
                    [P, kch * dup * C], F16, tag="s", name=f"s{nm}c{c0}"
                )
                sv = s[:, : w * dup * C].rearrange(
                    "p (k d c) -> p k d c", d=dup, c=C
                )
                nc.vector.tensor_tensor(out=sv, in0=ms[0], in1=ms[1], op=ADD)
                nc.vector.tensor_tensor(out=sv, in0=sv, in1=ms[2], op=ADD)
                ga = gq[:, :, :, 0:1].squeeze(3)
                if dup == 2:
                    ga = ga.unsqueeze(2).to_broadcast([P, w, 2, C])
                else:
                    ga = ga.unsqueeze(2)
                nc.vector.tensor_tensor(out=o4[:, c0:c1], in0=sv, in1=ga, op=ADD)

            nc.sync.dma_start(out=out_ap[r0 : r0 + P, :], in_=o[:])

        for rep in range(repeat):
            for lb in range(BPC):
                for sub in range(SUBS):
                    region(lb, sub, "A", rep)
                for sub in range(SUBS):
                    region(lb, sub, "C", rep)

    nc.compile()
    return nc


_nc_cache = {}


def _get_program(capA: int, capC: int, repeat: int = 1):
    key = (capA, capC, repeat)
    if key not in _nc_cache:
        _nc_cache[key] = build_program(capA, capC, repeat)
    return _nc_cache[key]


def _make_table(params: np.ndarray) -> np.ndarray:
    """fp16 diff-quad table [H*W, 128]: cell = 32 ch x (a, dx, dy, dxy)."""
    v = np.ascontiguousarray(np.transpose(params[0], (1, 2, 0))).astype(np.float32)
    vx = np.concatenate([v[:, 1:], v[:, -1:]], axis=1)
    vy = np.concatenate([v[1:], v[-1:]], axis=0)
    vxy = np.concatenate([vx[1:], vx[-1:]], axis=0)
    quad = np.stack([v, vx - v, vy - v, vxy - vx - vy + v, ], axis=-1)
    return quad.astype(np.float16).reshape(H * W, QUAD)


def _wrap_idx(arr):
    """[NSUB, cap] int16 -> [NSUB*128, cap//16] wrapped + 8x replicated."""
    nsub, cap = arr.shape
    sb_ = cap // 16
    iw = np.tile(arr.reshape(nsub, sb_, 16).transpose(0, 2, 1), (1, 8, 1))
    return np.ascontiguousarray(iw.reshape(nsub * 128, sb_))


def _host_prep(coord: np.ndarray, capA: int, capC: int):
    """Route queries to band-owner cores, pair queries per cell, build
    per-core device input tiles.  Returns per-core inputs + unshard maps."""
    xy = coord.astype(np.float32, copy=False)
    ix = (xy[:, 0] + np.float32(1.0)) * np.float32(0.5) * np.float32(W - 1)
    iy = (xy[:, 1] + np.float32(1.0)) * np.float32(0.5) * np.float32(H - 1)
    x0f = np.floor(ix)
    y0f = np.floor(iy)
    fx32 = ix - x0f
    fy32 = iy - y0f
    fx = fx32.astype(np.float16)
    fy = fy32.astype(np.float16)
    fxy = (fx32 * fy32).astype(np.float16)
    x0 = np.clip(x0f.astype(np.int32), 0, W - 1)
    y0 = np.clip(y0f.astype(np.int32), 0, H - 1)
    band = y0 >> 5
    cell = ((y0 & 31) << 10) | x0

    kdA, kdC = capA // 128, capC // 128
    NSUB = BPC * SUBS
    per_core = []
    unshard = []  # per core: (qidx_sorted, isA, row, col)
    for c in range(N_CORES):
        qi = np.nonzero((band >> 2) == c)[0]
        lb = (band[qi] - 4 * c).astype(np.int64)
        cl = cell[qi].astype(np.int64)
        skey = (lb << 15) | cl
        order = np.argsort(skey, kind="stable")
        qs = qi[order]
        sk = skey[order]
        lbs = lb[order]
        cls = cl[order]
        n = len(qs)

        newrun = np.empty(n, bool)
        newrun[0] = True
        np.not_equal(sk[1:], sk[:-1], out=newrun[1:])
        starts = np.nonzero(newrun)[0]
        runid = np.cumsum(newrun) - 1
        pos = np.arange(n) - starts[runid]
        rl = np.diff(np.append(starts, n))
        L = rl[runid]
        is_single = (pos == L - 1) & (L % 2 == 1)
        e = (pos & 1).astype(np.int64)
        pairstart = (~is_single) & (e == 0)

        descA_counts = np.bincount(lbs[pairstart], minlength=BPC)
        descC_counts = np.bincount(lbs[is_single], minlength=BPC)
        if descA_counts.max() > SUBS * capA or descC_counts.max() > SUBS * capC:
            return None, (int(descA_counts.max()), int(descC_counts.max()))

        offA = np.concatenate([[0], np.cumsum(descA_counts)[:-1]])
        offC = np.concatenate([[0], np.cumsum(descC_counts)[:-1]])
        # desc index within band, at flagged positions
        cumA = np.cumsum(pairstart) - 1
        cumC = np.cumsum(is_single) - 1
        dA = cumA - offA[lbs]  # valid where pairstart
        dC = cumC - offC[lbs]  # valid where is_single
        # propagate pair desc to the e=1 member (adjacent position)
        dA_q = dA.copy()
        dA_q[1:][e[1:] == 1] = dA[:-1][e[1:] == 1]

        isA = ~is_single
        # A mapping
        subA = dA_q // capA
        wA = dA_q % capA
        rowA = (lbs * SUBS + subA) * P + (wA % P)
        colA = (wA // P) * 2 + e
        # C mapping
        subC = dC // capC
        wC = dC % capC
        rowC = (lbs * SUBS + subC) * P + (wC % P)
        colC = wC // P

        row = np.where(isA, rowA, rowC)
        col = np.where(isA, colA, colC)
        unshard.append((qs, isA, row, col))

        # input tiles
        idxA_arr = np.zeros((NSUB, capA), np.int16)
        idxC_arr = np.zeros((NSUB, capC), np.int16)
        pstart = np.nonzero(pairstart)[0]
        single = np.nonzero(is_single)[0]
        rbA = (lbs[pstart] * SUBS + subA[pstart]).astype(np.int64)
        idxA_arr[rbA, wA[pstart]] = cls[pstart].astype(np.int16)
        rbC = (lbs[single] * SUBS + subC[single]).astype(np.int64)
        idxC_arr[rbC, wC[single]] = cls[single].astype(np.int16)

        fA_arr = np.zeros((NSUB, P, kdA, 2, 3), np.float16)
        fC_arr = np.zeros((NSUB, P, kdC, 3), np.float16)
        qA = np.nonzero(isA)[0]
        qC = single
        fvals = np.stack([fx, fy, fxy], axis=-1)  # [N, 3]
        rb = (lbs[qA] * SUBS + subA[qA]).astype(np.int64)
        fA_arr[rb, wA[qA] % P, wA[qA] // P, e[qA]] = fvals[qs[qA]]
        rbc = (lbs[qC] * SUBS + subC[qC]).astype(np.int64)
        fC_arr[rbc, wC[qC] % P, wC[qC] // P] = fvals[qs[qC]]

        per_core.append(
            {
                "idxA": _wrap_idx(idxA_arr),
                "idxC": _wrap_idx(idxC_arr),
                "fA": np.ascontiguousarray(fA_arr.reshape(NSUB * P, kdA * 2 * 3)),
                "fC": np.ascontiguousarray(fC_arr.reshape(NSUB * P, kdC * 3)),
            }
        )
    return (per_core, unshard), None


def _unshard(results, unshard, capA: int, capC: int) -> np.ndarray:
    kdA, kdC = capA // 128, capC // 128
    NSUB = BPC * SUBS
    out = np.empty((N_POINTS, C), np.float32)
    for c in range(N_CORES):
        qs, isA, row, col = unshard[c]
        oA = results[c]["outA"].reshape(NSUB * P, kdA * 2, C)
        oC = results[c]["outC"].reshape(NSUB * P, kdC, C)
        vals = np.empty((len(qs), C), np.float32)
        a = np.nonzero(isA)[0]
        b = np.nonzero(~isA)[0]
        vals[a] = oA[row[a], col[a]]
        vals[b] = oC[row[b], col[b]]
        out[qs] = vals
    return out


def _run(coord: np.ndarray, params: np.ndarray, trace: bool = False, **kw):
    assert coord.shape == (N_POINTS, 2) and params.shape == (1, C, H, W)
    capA, capC = CAP_A, CAP_C
    full_table = _make_table(params)
    while True:
        prep, maxes = _host_prep(coord, capA, capC)
        if prep is not None:
            break
        mA, mC = maxes
        capA = max(capA, ((mA // SUBS + 255) // 128) * 128)
        capC = max(capC, ((mC // SUBS + 255) // 128) * 128)
    per_core, unshard = prep
    nc = _get_program(capA, capC)
    in_maps = []
    for c in range(N_CORES):
        tbl = full_table[4 * c * BAND_CELLS : 4 * (c + 1) * BAND_CELLS]
        in_maps.append({"table": np.ascontiguousarray(tbl), **per_core[c]})
    res = run_bass_kernel_spmd(nc, in_maps, list(range(N_CORES)), trace=trace, **kw)
    return _unshard(res.results, unshard, capA, capC), res


def kernel(coord: np.ndarray, params: np.ndarray) -> np.ndarray:
    return _run(coord, params)[0]
